# revision 9
# baseline (speedup 1.0000x reference)
"""CenterlineDiceLoss (soft-skeleton clDice) Trainium2 Bass kernel.

Strategy: data-parallel over the batch (8 images -> 8 NeuronCores).  Each
core computes the two soft skeletons (sigmoid(pred), target) of its image
entirely SBUF-resident in fp16, using the identity that the erosion inside
``open(e_i)`` *is* ``e_{i+1}``, so each of the 11 rounds needs one 3x3 min
pool + one 3x3 max pool (separable, pairwise decomposition).  The skel
recurrence is tracked in complement space w = 1 - skel, which turns the
relu-laden update into  w *= (1 + (o - e))  (two fused scalar_tensor_tensor
ops), and the final four global sums reduce on-chip to 6 scalars per core
that the host combines into the loss.

Layout: image row 8p+j lives on partition p at free slot (j, c); all DVE
operands are kept 4B-aligned (shifted reads go through ScalarE copies) so
fp16 tensor_tensor runs in the 2x perf mode.  Vertical pooling crosses
partitions only at the 2 boundary rows per partition, exchanged with small
SBUF->SBUF DMAs.
"""

import os
import numpy as np

NUM_ITER = 10
SMOOTH = 1.0
EPS = 1e-7
SENT = 30000.0  # pad sentinel (exactly representable in fp16)

_BUILT = {}


def _install_walrus_wait_patch():
    """This container's walrus rejects >1 sync-wait per instruction; split
    extra waits onto NoOp/Drain instructions on the same engine."""
    import concourse.tile as tile_mod
    import mybir

    if getattr(tile_mod.TileContext, "_cldice_patched", False):
        return

    _orig_add_instruction = tile_mod.TileContext._add_instruction
    _ctr = [0]

    def _patched_add_instruction(self, inst):
        si = getattr(inst, "sync_info", None)
        if (
            si is not None
            and si.on_wait is not None
            and len(si.on_wait) > 1
            and inst.engine != mybir.EngineType.Unassigned
        ):
            waits = list(si.on_wait)
            ups = list(si.on_update) if si.on_update else []
            for w in waits[:-1]:
                _ctr[0] += 1
                nop = mybir.InstNoOp(
                    name=f"{inst.name}_sw{_ctr[0]}",
                    sync_info=mybir.SyncInfo(on_wait=[w], on_update=[]),
                    bass_nofuse=True,
                    engine=inst.engine,
                )
                _orig_add_instruction(self, nop)
            inst.sync_info = mybir.SyncInfo(on_wait=waits[-1:], on_update=ups)
        return _orig_add_instruction(self, inst)

    def _patched_drain_and_barrier(self, tick_clock, wait_clock):
        nc = self.nc
        drain_inst = nc.sync.drain()
        wait_clock.add_sem_waits(
            drain_inst.ins, tile_mod.ScopedClock({None: tick_clock.global_clock})
        )
        si = drain_inst.ins.sync_info
        if si is not None and si.on_wait is not None and len(si.on_wait) > 1:
            waits = list(si.on_wait)
            ups = list(si.on_update) if si.on_update else []
            drain_inst.ins.sync_info = mybir.SyncInfo(on_wait=waits[:1], on_update=[])
            for w in waits[1:]:
                extra = nc.sync.drain()
                extra.ins.sync_info = mybir.SyncInfo(on_wait=[w], on_update=[])
            if ups:
                extra2 = nc.sync.drain()
                extra2.ins.sync_info = mybir.SyncInfo(on_wait=[], on_update=ups)
        nc.all_engine_barrier()
        assert self.sems is not None
        popped = nc._tile_sem_poison_stack.pop()
        assert popped is self._sem_poison
        nc.clear_and_free_semaphores(list(self.sems.allocated().values()))
        nc.all_engine_barrier()

    tile_mod.TileContext._add_instruction = _patched_add_instruction
    tile_mod.TileContext._drain_and_barrier = _patched_drain_and_barrier
    tile_mod.TileContext._cldice_patched = True


def build_nc(H=1024, W=1024, dtname="fp16", rounds=NUM_ITER + 1):
    """Build the single-core Bass program (run SPMD across 8 cores)."""
    import concourse.bass as bass
    import concourse.bass_isa as bass_isa
    import concourse.tile as tile
    import mybir

    _install_walrus_wait_patch()

    P = 128
    R = H // P          # image rows per partition
    WB = W + 4          # padded row width (2 sentinel cols each side)
    T = W // 2          # column strip width
    NS = W // T
    fp32 = mybir.dt.float32
    dt = {"fp16": mybir.dt.float16, "fp32": mybir.dt.float32}[dtname]
    AL = mybir.AluOpType
    AF = mybir.ActivationFunctionType
    use_shift_copies = dtname == "fp16"

    nc = bass.Bass("TRN2", target_bir_lowering=False, debug=False)
    pred_d = nc.dram_tensor("pred", [H, W], fp32, kind="ExternalInput").ap()
    targ_d = nc.dram_tensor("target", [H, W], fp32, kind="ExternalInput").ap()
    p16_d = nc.dram_tensor("p16", [P, R, W], dt).ap()
    t16_d = nc.dram_tensor("t16", [P, R, W], dt).ap()
    out_d = nc.dram_tensor("out", [1, 16], fp32, kind="ExternalOutput").ap()

    with tile.TileContext(nc) as tc:
        with tc.tile_pool(name="persist", bufs=1) as pp:
            # persistent state, per chain (0 = pred-prob skeleton, 1 = target)
            eA, eB, wbuf = [], [], []
            for ch in range(2):
                eA.append(pp.tile([P, R, WB], dt, tag=f"eA{ch}", name=f"eA{ch}"))
                eB.append(pp.tile([P, R, WB], dt, tag=f"eB{ch}", name=f"eB{ch}"))
                wbuf.append(pp.tile([P, R, W], dt, tag=f"w{ch}", name=f"w{ch}"))
            consts = pp.tile([P, 2, T], dt, tag="consts")  # 0: +SENT, 1: -SENT
            accs = pp.tile([P, 16], fp32, tag="accs")
            redout = pp.tile([P, 16], fp32, tag="redout")

            ones = pp.tile([P, 1], fp32, tag="ones", name="ones")
            nc.vector.memset(ones[:], 1.0)
            nc.vector.memset(accs[:], 0.0)
            nc.vector.memset(consts[:, 0:1, :], SENT)
            nc.vector.memset(consts[:, 1:2, :], -SENT)

            # ---------------- init: load f32, sigmoid/cast to fp16 ----------
            # strip-wise to keep the f32 staging small; accumulate E/F sums.
            with tc.tile_pool(name="init", bufs=2) as ip:
                for ch, (src_d, func, acc_col) in enumerate(
                    [(pred_d, AF.Sigmoid, 5), (targ_d, AF.Copy, 4)]
                ):
                    src_r = src_d.rearrange("(p j) c -> p j c", p=P)
                    for s in range(NS):
                        cs = T * s
                        tmp32 = ip.tile([P, R, T], fp32, tag="tmp32")
                        nc.sync.dma_start(tmp32[:], src_r[:, :, cs : cs + T])
                        col = acc_col if s == NS - 1 else 8 + s
                        nc.scalar.activation(
                            eA[ch][:, :, cs + 2 : cs + T + 2],
                            tmp32[:],
                            func,
                            accum_out=accs[:, col : col + 1],
                        )
                    # combine strip partials into the final E/F column
                    for s in range(NS - 1):
                        nc.vector.tensor_tensor(
                            out=accs[:, acc_col : acc_col + 1],
                            in0=accs[:, acc_col : acc_col + 1],
                            in1=accs[:, 8 + s : 9 + s],
                            op=AL.add,
                        )
                    dst_d = p16_d if ch == 0 else t16_d
                    nc.sync.dma_start(dst_d[:], eA[ch][:, :, 2 : W + 2])
                    # sentinel pads: eA starts as min-source (+S); eB's first
                    # role is max-source (-S)
                    nc.vector.memset(eA[ch][:, :, 0:2], SENT)
                    nc.vector.memset(eA[ch][:, :, W + 2 : W + 4], SENT)
                    nc.vector.memset(eB[ch][:, :, 0:2], -SENT)
                    nc.vector.memset(eB[ch][:, :, W + 2 : W + 4], -SENT)

            with tc.tile_pool(name="scr", bufs=2) as scr:

                # ---------------- pool pass helper --------------------------
                def pool_pass(op, src, dst_of_strip, cidx):
                    """3x3 min/max pool of `src` (padded [P,R,WB]).
                    dst_of_strip(s) -> output AP [P,R,T] for strip s.
                    cidx: 0 for min (+S halo edge), 1 for max (-S)."""
                    for s in range(NS):
                        cs = T * s
                        if use_shift_copies:
                            xs = scr.tile([P, R, T + 2], dt, tag="xs")
                            nc.scalar.activation(
                                xs[:], src[:, :, cs + 1 : cs + T + 3], AF.Copy
                            )
                            in1_h1 = xs[:]
                        else:
                            in1_h1 = src[:, :, cs + 1 : cs + T + 3]
                        m1 = scr.tile([P, R, T + 2], dt, tag="m1")
                        nc.vector.tensor_tensor(
                            out=m1[:],
                            in0=src[:, :, cs : cs + T + 2],
                            in1=in1_h1,
                            op=op,
                        )
                        if use_shift_copies:
                            m1s = scr.tile([P, R, T], dt, tag="m1s")
                            nc.scalar.activation(m1s[:], m1[:, :, 1 : T + 1], AF.Copy)
                            in1_h2 = m1s[:]
                        else:
                            in1_h2 = m1[:, :, 1 : T + 1]
                        h = scr.tile([P, R + 2, T], dt, tag="h")
                        nc.vector.tensor_tensor(
                            out=h[:, 1 : R + 1, :],
                            in0=m1[:, :, 2 : T + 2],
                            in1=in1_h2,
                            op=op,
                        )
                        # row halo exchange across partitions
                        nc.sync.dma_start(h[1:P, 0:1, :], h[0 : P - 1, R : R + 1, :])
                        nc.sync.dma_start(
                            h[0:1, 0:1, :], consts[0:1, cidx : cidx + 1, :]
                        )
                        nc.sync.dma_start(
                            h[0 : P - 1, R + 1 : R + 2, :], h[1:P, 1:2, :]
                        )
                        nc.sync.dma_start(
                            h[P - 1 : P, R + 1 : R + 2, :],
                            consts[0:1, cidx : cidx + 1, :],
                        )
                        m2 = scr.tile([P, R + 1, T], dt, tag="m2")
                        nc.vector.tensor_tensor(
                            out=m2[:],
                            in0=h[:, 0 : R + 1, :],
                            in1=h[:, 1 : R + 2, :],
                            op=op,
                        )
                        nc.vector.tensor_tensor(
                            out=dst_of_strip(s),
                            in0=m2[:, 0:R, :],
                            in1=m2[:, 1 : R + 1, :],
                            op=op,
                        )

                # ---------------- skeleton rounds ---------------------------
                cur = [eA[0], eA[1]]
                nxt = [eB[0], eB[1]]
                for i in range(rounds):
                    for ch in range(2):
                        pool_pass(
                            AL.min,
                            cur[ch],
                            lambda s, ch=ch: nxt[ch][
                                :, :, T * s + 2 : T * s + T + 2
                            ],
                            0,
                        )
                    for ch in range(2):
                        o_strips = [None] * NS

                        def max_dst(s, o_strips=o_strips):
                            o = scr.tile([P, R, T], dt, tag="o", name="o")
                            o_strips[s] = o
                            return o[:]

                        pool_pass(AL.max, nxt[ch], max_dst, 1)
                        # nxt's pads served the max pass (-S); flip to +S for
                        # its next life as min-source
                        nc.gpsimd.memset(nxt[ch][:, :, 0:2], SENT)
                        nc.gpsimd.memset(nxt[ch][:, :, W + 2 : W + 4], SENT)
                        for s in range(NS):
                            cs = T * s
                            t_s = scr.tile([P, R, T], dt, tag="xs")
                            nc.vector.scalar_tensor_tensor(
                                out=t_s[:],
                                in0=cur[ch][:, :, cs + 2 : cs + T + 2],
                                scalar=-1.0,
                                op0=AL.mult,
                                in1=o_strips[s][:],
                                op1=AL.add,
                            )
                            if i == 0:
                                nc.scalar.activation(
                                    wbuf[ch][:, :, cs : cs + T],
                                    t_s[:],
                                    AF.Copy,
                                    bias=1.0,
                                )
                            else:
                                nc.vector.scalar_tensor_tensor(
                                    out=wbuf[ch][:, :, cs : cs + T],
                                    in0=t_s[:],
                                    scalar=1.0,
                                    op0=AL.add,
                                    in1=wbuf[ch][:, :, cs : cs + T],
                                    op1=AL.mult,
                                )
                        # cur's pads (+S, was min-source) -> -S for its next
                        # life as max-source (it becomes nxt after the swap)
                        if i < rounds - 1:
                            nc.gpsimd.memset(cur[ch][:, :, 0:2], -SENT)
                            nc.gpsimd.memset(cur[ch][:, :, W + 2 : W + 4], -SENT)
                    cur, nxt = nxt, cur

                # ---------------- final sums --------------------------------
                # accs cols: 0:A=sum(w_p*t16) 1:B=sum(w_p) 2:C=sum(w_t*p16)
                #            3:D=sum(w_t)     4:E=sum(t16) 5:F=sum(p16)
                for ch in range(2):
                    other_d = t16_d if ch == 0 else p16_d
                    prod_col = 0 if ch == 0 else 2
                    wsum_col = 1 if ch == 0 else 3
                    pc = [8 + 4 * ch + s for s in range(NS)]
                    wc = [10 + 4 * ch + s for s in range(NS)]
                    for s in range(NS):
                        cs = T * s
                        ob = scr.tile([P, R, T], dt, tag="h")
                        nc.sync.dma_start(ob[:], other_d[:, :, cs : cs + T])
                        junk = scr.tile([P, R, T], dt, tag="m2")
                        nc.vector.tensor_tensor(
                            out=junk[:],
                            in0=wbuf[ch][:, :, cs : cs + T],
                            in1=ob[:],
                            op=AL.mult,
                        )
                        nc.vector.tensor_reduce(
                            out=accs[:, pc[s] : pc[s] + 1],
                            in_=junk[:],
                            axis=mybir.AxisListType.XY,
                            op=AL.add,
                        )
                        junk2 = scr.tile([P, R, T], dt, tag="o")
                        nc.scalar.activation(
                            junk2[:],
                            wbuf[ch][:, :, cs : cs + T],
                            AF.Copy,
                            accum_out=accs[:, wc[s] : wc[s] + 1],
                        )
                    nc.vector.tensor_tensor(
                        out=accs[:, prod_col : prod_col + 1],
                        in0=accs[:, pc[0] : pc[0] + 1],
                        in1=accs[:, pc[1] : pc[1] + 1],
                        op=AL.add,
                    )
                    nc.vector.tensor_tensor(
                        out=accs[:, wsum_col : wsum_col + 1],
                        in0=accs[:, wc[0] : wc[0] + 1],
                        in1=accs[:, wc[1] : wc[1] + 1],
                        op=AL.add,
                    )

                with tc.tile_pool(name="psum", bufs=1, space="PSUM") as psp:
                    ps = psp.tile([1, 16], fp32, name="ps")
                    nc.tensor.matmul(ps[:], ones[:], accs[:], start=True, stop=True)
                    nc.vector.tensor_copy(redout[0:1, :], ps[:])
                nc.sync.dma_start(out_d[:], redout[0:1, :])

    return nc


def _get_built(H=1024, W=1024, dtname=None):
    if dtname is None:
        dtname = os.environ.get("CLDICE_DT", "fp16")
    key = (H, W, dtname)
    if key not in _BUILT:
        _BUILT[key] = build_nc(H, W, dtname)
    return _BUILT[key]


_last_run_wall = [None]


def kernel(pred: np.ndarray, target: np.ndarray) -> np.ndarray:
    """Full-input entry point: pred/target [8,1,1024,1024] f32 -> scalar."""
    import time
    from concourse.bass_utils import run_bass_kernel_spmd

    n_cores = pred.shape[0]
    nc = _get_built(pred.shape[2], pred.shape[3])
    in_maps = [
        {
            "pred": np.ascontiguousarray(pred[c, 0], dtype=np.float32),
            "target": np.ascontiguousarray(target[c, 0], dtype=np.float32),
        }
        for c in range(n_cores)
    ]
    t0 = time.time()
    res = run_bass_kernel_spmd(nc, in_maps, list(range(n_cores)))
    _last_run_wall[0] = time.time() - t0
    outs = np.stack([res.results[c]["out"][0] for c in range(n_cores)])  # [8,16]
    return _combine(outs, pred.shape[2] * pred.shape[3])


def _combine(outs: np.ndarray, n_per_core: int) -> np.ndarray:
    o = outs.astype(np.float64)
    A, B, C, D, E, F = (o[:, k] for k in range(6))
    S1 = np.sum(E - A)  # sum(skel_pred * target)
    S2 = np.sum(n_per_core - B)  # sum(skel_pred)
    S3 = np.sum(F - C)  # sum(skel_target * pred_prob)
    S4 = np.sum(n_per_core - D)  # sum(skel_target)
    tprec = (S1 + SMOOTH) / (S2 + SMOOTH)
    tsens = (S3 + SMOOTH) / (S4 + SMOOTH)
    cl_dice = 2.0 * tprec * tsens / (tprec + tsens + EPS)
    return np.float32(1.0 - cl_dice)


# revision 12
# speedup vs baseline: 1.7833x; 1.7833x over previous
"""CenterlineDiceLoss (soft-skeleton clDice) Trainium2 Bass kernel.

Strategy: data-parallel over the batch (8 images -> 8 NeuronCores).  Each
core computes the two soft skeletons (sigmoid(pred), target) of its image
entirely SBUF-resident in fp16, using the identity that the erosion inside
``open(e_i)`` *is* ``e_{i+1}``, so each of the 11 rounds needs one 3x3 min
pool + one 3x3 max pool (separable, pairwise decomposition).  The skel
recurrence is tracked in complement space w = 1 - skel, which turns the
relu-laden update into  w *= (1 + (o - e))  (two fused scalar_tensor_tensor
ops), and the final four global sums reduce on-chip to 6 scalars per core
that the host combines into the loss.

Layout: image row 8p+j lives on partition p at free slot (j, c); all DVE
operands are kept 4B-aligned (shifted reads go through ScalarE copies) so
fp16 tensor_tensor runs in the 2x perf mode.  Vertical pooling crosses
partitions only at the 2 boundary rows per partition, exchanged with small
SBUF->SBUF DMAs.
"""

import os
import numpy as np

NUM_ITER = 10
SMOOTH = 1.0
EPS = 1e-7
SENT = 30000.0  # pad sentinel (exactly representable in fp16)

_BUILT = {}


def _install_walrus_wait_patch():
    """This container's walrus rejects >1 sync-wait per instruction; split
    extra waits onto NoOp/Drain instructions on the same engine."""
    import concourse.tile as tile_mod
    import mybir

    if getattr(tile_mod.TileContext, "_cldice_patched", False):
        return

    _orig_add_instruction = tile_mod.TileContext._add_instruction
    _ctr = [0]

    def _patched_add_instruction(self, inst):
        si = getattr(inst, "sync_info", None)
        if (
            si is not None
            and si.on_wait is not None
            and len(si.on_wait) > 1
            and inst.engine != mybir.EngineType.Unassigned
        ):
            waits = list(si.on_wait)
            ups = list(si.on_update) if si.on_update else []
            for w in waits[:-1]:
                _ctr[0] += 1
                nop = mybir.InstNoOp(
                    name=f"{inst.name}_sw{_ctr[0]}",
                    sync_info=mybir.SyncInfo(on_wait=[w], on_update=[]),
                    bass_nofuse=True,
                    engine=inst.engine,
                )
                _orig_add_instruction(self, nop)
            inst.sync_info = mybir.SyncInfo(on_wait=waits[-1:], on_update=ups)
        return _orig_add_instruction(self, inst)

    def _patched_drain_and_barrier(self, tick_clock, wait_clock):
        nc = self.nc
        drain_inst = nc.sync.drain()
        wait_clock.add_sem_waits(
            drain_inst.ins, tile_mod.ScopedClock({None: tick_clock.global_clock})
        )
        si = drain_inst.ins.sync_info
        if si is not None and si.on_wait is not None and len(si.on_wait) > 1:
            waits = list(si.on_wait)
            ups = list(si.on_update) if si.on_update else []
            drain_inst.ins.sync_info = mybir.SyncInfo(on_wait=waits[:1], on_update=[])
            for w in waits[1:]:
                extra = nc.sync.drain()
                extra.ins.sync_info = mybir.SyncInfo(on_wait=[w], on_update=[])
            if ups:
                extra2 = nc.sync.drain()
                extra2.ins.sync_info = mybir.SyncInfo(on_wait=[], on_update=ups)
        nc.all_engine_barrier()
        assert self.sems is not None
        popped = nc._tile_sem_poison_stack.pop()
        assert popped is self._sem_poison
        nc.clear_and_free_semaphores(list(self.sems.allocated().values()))
        nc.all_engine_barrier()

    tile_mod.TileContext._add_instruction = _patched_add_instruction
    tile_mod.TileContext._drain_and_barrier = _patched_drain_and_barrier
    tile_mod.TileContext._cldice_patched = True


def build_nc(H=1024, W=1024, dtname="fp16", rounds=NUM_ITER + 1):
    """Build the single-core Bass program (run SPMD across 8 cores)."""
    import concourse.bass as bass
    import concourse.bass_isa as bass_isa
    import concourse.tile as tile
    import mybir

    _install_walrus_wait_patch()

    P = 128
    R = H // P          # image rows per partition
    WB = W + 4          # padded row width (2 sentinel cols each side)
    T = W // 2          # column strip width
    NS = W // T
    fp32 = mybir.dt.float32
    dt = {"fp16": mybir.dt.float16, "fp32": mybir.dt.float32}[dtname]
    AL = mybir.AluOpType
    AF = mybir.ActivationFunctionType
    use_shift_copies = bool(os.environ.get("CLDICE_SHIFTCOPY"))

    nc = bass.Bass("TRN2", target_bir_lowering=False, debug=False)
    pred_d = nc.dram_tensor("pred", [H, W], dt, kind="ExternalInput").ap()
    targ_d = nc.dram_tensor("target", [H, W], dt, kind="ExternalInput").ap()
    p16_d = nc.dram_tensor("p16", [P, R, W], dt).ap()
    t16_d = nc.dram_tensor("t16", [P, R, W], dt).ap()
    out_d = nc.dram_tensor("out", [1, 16], fp32, kind="ExternalOutput").ap()

    with tile.TileContext(nc) as tc:
        with tc.tile_pool(name="persist", bufs=1) as pp:
            # persistent state, per chain (0 = pred-prob skeleton, 1 = target)
            eA, eB, wbuf = [], [], []
            for ch in range(2):
                eA.append(pp.tile([P, R, WB], dt, tag=f"eA{ch}", name=f"eA{ch}"))
                eB.append(pp.tile([P, R, WB], dt, tag=f"eB{ch}", name=f"eB{ch}"))
                wbuf.append(pp.tile([P, R, W], dt, tag=f"w{ch}", name=f"w{ch}"))
            consts = pp.tile([P, 2, T], dt, tag="consts")  # 0: +SENT, 1: -SENT
            accs = pp.tile([P, 16], fp32, tag="accs")
            redout = pp.tile([P, 16], fp32, tag="redout")

            ones = pp.tile([P, 1], fp32, tag="ones", name="ones")
            nc.vector.memset(ones[:], 1.0)
            nc.vector.memset(accs[:], 0.0)
            nc.vector.memset(consts[:, 0:1, :], SENT)
            nc.vector.memset(consts[:, 1:2, :], -SENT)

            # ---------------- init: load f32, sigmoid/cast to fp16 ----------
            # strip-wise to keep the f32 staging small; accumulate E/F sums.
            with tc.tile_pool(name="init", bufs=2) as ip:
                for ch, (src_d, func, acc_col) in enumerate(
                    [(pred_d, AF.Sigmoid, 5), (targ_d, AF.Copy, 4)]
                ):
                    src_r = src_d.rearrange("(p j) c -> p j c", p=P)
                    for s in range(NS):
                        cs = T * s
                        tmp32 = ip.tile([P, R, T], dt, tag="tmp32")
                        nc.sync.dma_start(tmp32[:], src_r[:, :, cs : cs + T])
                        col = acc_col if s == NS - 1 else 8 + s
                        nc.scalar.activation(
                            eA[ch][:, :, cs + 2 : cs + T + 2],
                            tmp32[:],
                            func,
                            accum_out=accs[:, col : col + 1],
                        )
                    # combine strip partials into the final E/F column
                    for s in range(NS - 1):
                        nc.vector.tensor_tensor(
                            out=accs[:, acc_col : acc_col + 1],
                            in0=accs[:, acc_col : acc_col + 1],
                            in1=accs[:, 8 + s : 9 + s],
                            op=AL.add,
                        )
                    dst_d = p16_d if ch == 0 else t16_d
                    nc.sync.dma_start(dst_d[:], eA[ch][:, :, 2 : W + 2])
                    # sentinel pads: eA starts as min-source (+S); eB's first
                    # role is max-source (-S)
                    nc.vector.memset(eA[ch][:, :, 0:2], SENT)
                    nc.vector.memset(eA[ch][:, :, W + 2 : W + 4], SENT)
                    nc.vector.memset(eB[ch][:, :, 0:2], -SENT)
                    nc.vector.memset(eB[ch][:, :, W + 2 : W + 4], -SENT)

            with tc.tile_pool(name="scr", bufs=2) as scr:

                # ---------------- pool pass helper --------------------------
                def pool_pass(op, src, dst_of_strip, cidx):
                    """3x3 min/max pool of `src` (padded [P,R,WB]).
                    dst_of_strip(s) -> output AP [P,R,T] for strip s.
                    cidx: 0 for min (+S halo edge), 1 for max (-S)."""
                    for s in range(NS):
                        cs = T * s
                        if use_shift_copies:
                            xs = scr.tile([P, R, T + 2], dt, tag="xs")
                            nc.scalar.activation(
                                xs[:], src[:, :, cs + 1 : cs + T + 3], AF.Copy
                            )
                            in1_h1 = xs[:]
                        else:
                            in1_h1 = src[:, :, cs + 1 : cs + T + 3]
                        m1 = scr.tile([P, R, T + 2], dt, tag="m1")
                        nc.vector.tensor_tensor(
                            out=m1[:],
                            in0=src[:, :, cs : cs + T + 2],
                            in1=in1_h1,
                            op=op,
                        )
                        if use_shift_copies:
                            m1s = scr.tile([P, R, T], dt, tag="m1s")
                            nc.scalar.activation(m1s[:], m1[:, :, 1 : T + 1], AF.Copy)
                            in1_h2 = m1s[:]
                        else:
                            in1_h2 = m1[:, :, 1 : T + 1]
                        h = scr.tile([P, R + 2, T], dt, tag="h")
                        nc.vector.tensor_tensor(
                            out=h[:, 1 : R + 1, :],
                            in0=m1[:, :, 2 : T + 2],
                            in1=in1_h2,
                            op=op,
                        )
                        # row halo exchange across partitions
                        if os.environ.get("CLDICE_NO_HALO"):
                            nc.vector.memset(h[:, 0:1, :], 0.0)
                            nc.vector.memset(h[:, R + 1 : R + 2, :], 0.0)
                            m2 = scr.tile([P, R + 1, T], dt, tag="m2")
                            nc.vector.tensor_tensor(
                                out=m2[:],
                                in0=h[:, 0 : R + 1, :],
                                in1=h[:, 1 : R + 2, :],
                                op=op,
                            )
                            nc.vector.tensor_tensor(
                                out=dst_of_strip(s),
                                in0=m2[:, 0:R, :],
                                in1=m2[:, 1 : R + 1, :],
                                op=op,
                            )
                            continue
                        nc.sync.dma_start(h[1:P, 0:1, :], h[0 : P - 1, R : R + 1, :])
                        nc.sync.dma_start(
                            h[0:1, 0:1, :], consts[0:1, cidx : cidx + 1, :]
                        )
                        nc.sync.dma_start(
                            h[0 : P - 1, R + 1 : R + 2, :], h[1:P, 1:2, :]
                        )
                        nc.sync.dma_start(
                            h[P - 1 : P, R + 1 : R + 2, :],
                            consts[0:1, cidx : cidx + 1, :],
                        )
                        m2 = scr.tile([P, R + 1, T], dt, tag="m2")
                        nc.vector.tensor_tensor(
                            out=m2[:],
                            in0=h[:, 0 : R + 1, :],
                            in1=h[:, 1 : R + 2, :],
                            op=op,
                        )
                        nc.vector.tensor_tensor(
                            out=dst_of_strip(s),
                            in0=m2[:, 0:R, :],
                            in1=m2[:, 1 : R + 1, :],
                            op=op,
                        )

                # ---------------- skeleton rounds ---------------------------
                cur = [eA[0], eA[1]]
                nxt = [eB[0], eB[1]]
                for i in range(rounds):
                    for ch in range(2):
                        pool_pass(
                            AL.min,
                            cur[ch],
                            lambda s, ch=ch: nxt[ch][
                                :, :, T * s + 2 : T * s + T + 2
                            ],
                            0,
                        )
                    for ch in range(2):
                        o_strips = [None] * NS

                        def max_dst(s, o_strips=o_strips):
                            o = scr.tile([P, R, T], dt, tag="o", name="o")
                            o_strips[s] = o
                            return o[:]

                        pool_pass(AL.max, nxt[ch], max_dst, 1)
                        # nxt's pads served the max pass (-S); flip to +S for
                        # its next life as min-source
                        nc.gpsimd.memset(nxt[ch][:, :, 0:2], SENT)
                        nc.gpsimd.memset(nxt[ch][:, :, W + 2 : W + 4], SENT)
                        for s in range(NS):
                            cs = T * s
                            t_s = scr.tile([P, R, T], dt, tag="xs")
                            nc.vector.scalar_tensor_tensor(
                                out=t_s[:],
                                in0=cur[ch][:, :, cs + 2 : cs + T + 2],
                                scalar=-1.0,
                                op0=AL.mult,
                                in1=o_strips[s][:],
                                op1=AL.add,
                            )
                            if i == 0:
                                nc.scalar.activation(
                                    wbuf[ch][:, :, cs : cs + T],
                                    t_s[:],
                                    AF.Copy,
                                    bias=1.0,
                                )
                            else:
                                nc.vector.scalar_tensor_tensor(
                                    out=wbuf[ch][:, :, cs : cs + T],
                                    in0=t_s[:],
                                    scalar=1.0,
                                    op0=AL.add,
                                    in1=wbuf[ch][:, :, cs : cs + T],
                                    op1=AL.mult,
                                )
                        # cur's pads (+S, was min-source) -> -S for its next
                        # life as max-source (it becomes nxt after the swap)
                        if i < rounds - 1:
                            nc.gpsimd.memset(cur[ch][:, :, 0:2], -SENT)
                            nc.gpsimd.memset(cur[ch][:, :, W + 2 : W + 4], -SENT)
                    cur, nxt = nxt, cur

                # ---------------- final sums --------------------------------
                # accs cols: 0:A=sum(w_p*t16) 1:B=sum(w_p) 2:C=sum(w_t*p16)
                #            3:D=sum(w_t)     4:E=sum(t16) 5:F=sum(p16)
                for ch in range(2):
                    other_d = t16_d if ch == 0 else p16_d
                    prod_col = 0 if ch == 0 else 2
                    wsum_col = 1 if ch == 0 else 3
                    pc = [8 + 4 * ch + s for s in range(NS)]
                    wc = [10 + 4 * ch + s for s in range(NS)]
                    for s in range(NS):
                        cs = T * s
                        ob = scr.tile([P, R, T], dt, tag="h")
                        nc.sync.dma_start(ob[:], other_d[:, :, cs : cs + T])
                        junk = scr.tile([P, R, T], dt, tag="m2")
                        nc.vector.tensor_tensor(
                            out=junk[:],
                            in0=wbuf[ch][:, :, cs : cs + T],
                            in1=ob[:],
                            op=AL.mult,
                        )
                        nc.vector.tensor_reduce(
                            out=accs[:, pc[s] : pc[s] + 1],
                            in_=junk[:],
                            axis=mybir.AxisListType.XY,
                            op=AL.add,
                        )
                        junk2 = scr.tile([P, R, T], dt, tag="o")
                        nc.scalar.activation(
                            junk2[:],
                            wbuf[ch][:, :, cs : cs + T],
                            AF.Copy,
                            accum_out=accs[:, wc[s] : wc[s] + 1],
                        )
                    nc.vector.tensor_tensor(
                        out=accs[:, prod_col : prod_col + 1],
                        in0=accs[:, pc[0] : pc[0] + 1],
                        in1=accs[:, pc[1] : pc[1] + 1],
                        op=AL.add,
                    )
                    nc.vector.tensor_tensor(
                        out=accs[:, wsum_col : wsum_col + 1],
                        in0=accs[:, wc[0] : wc[0] + 1],
                        in1=accs[:, wc[1] : wc[1] + 1],
                        op=AL.add,
                    )

                with tc.tile_pool(name="psum", bufs=1, space="PSUM") as psp:
                    ps = psp.tile([1, 16], fp32, name="ps")
                    nc.tensor.matmul(ps[:], ones[:], accs[:], start=True, stop=True)
                    nc.vector.tensor_copy(redout[0:1, :], ps[:])
                nc.sync.dma_start(out_d[:], redout[0:1, :])

    return nc


def _get_built(H=1024, W=1024, dtname=None):
    if dtname is None:
        dtname = os.environ.get("CLDICE_DT", "fp16")
    key = (H, W, dtname)
    if key not in _BUILT:
        _BUILT[key] = build_nc(H, W, dtname)
    return _BUILT[key]


_last_run_wall = [None]


def kernel(pred: np.ndarray, target: np.ndarray) -> np.ndarray:
    """Full-input entry point: pred/target [8,1,1024,1024] f32 -> scalar."""
    import time
    from concourse.bass_utils import run_bass_kernel_spmd

    n_cores = pred.shape[0]
    dtname = os.environ.get("CLDICE_DT", "fp16")
    _np_in_dt = np.float16 if dtname == "fp16" else np.float32
    nc = _get_built(pred.shape[2], pred.shape[3], dtname)
    in_maps = [
        {
            "pred": np.ascontiguousarray(pred[c, 0], dtype=_np_in_dt),
            "target": np.ascontiguousarray(target[c, 0], dtype=_np_in_dt),
        }
        for c in range(n_cores)
    ]
    t0 = time.time()
    res = run_bass_kernel_spmd(nc, in_maps, list(range(n_cores)))
    _last_run_wall[0] = time.time() - t0
    outs = np.stack([res.results[c]["out"][0] for c in range(n_cores)])  # [8,16]
    return _combine(outs, pred.shape[2] * pred.shape[3])


def _combine(outs: np.ndarray, n_per_core: int) -> np.ndarray:
    o = outs.astype(np.float64)
    A, B, C, D, E, F = (o[:, k] for k in range(6))
    S1 = np.sum(E - A)  # sum(skel_pred * target)
    S2 = np.sum(n_per_core - B)  # sum(skel_pred)
    S3 = np.sum(F - C)  # sum(skel_target * pred_prob)
    S4 = np.sum(n_per_core - D)  # sum(skel_target)
    tprec = (S1 + SMOOTH) / (S2 + SMOOTH)
    tsens = (S3 + SMOOTH) / (S4 + SMOOTH)
    cl_dice = 2.0 * tprec * tsens / (tprec + tsens + EPS)
    return np.float32(1.0 - cl_dice)
